# revision 26
# baseline (speedup 1.0000x reference)
"""ExtractSearchWindows Trainium2 kernel (8 NeuronCores, Bass/Tile).

out[b, h, w, dy*cv+dx, ky*8+kx] = uint8(P[b, h+off+dy+ky, w+off+dx+kx])
with P = zero-pad(inputs[:, 0], 7) and off = 3 - search_range.

The output (196.6 MB u8) is a pure byte-replication of a tiny input, so
the kernel is bound by SBUF-AXI / SDMA write bandwidth (16 engines x
~27.2 GB/s = ~435 GB/s per core).  Work is sharded over (b, h): each of
the 8 cores produces 48 output rows as 384 segments (segment = 40-pixel
row chunk) in 3 tiles of 128 partitions.

Expansion uses dy-fused per-pixel DVE copies: one 4-dim tensor_copy per
output pixel covers all (dy, ky, dx, kx) at once by exploiting that dy
and ky address source rows with the SAME stride (overlapping reads),
400 u32 per partition per copy.  DVE issue rate ~50ns + 0.62ns/elem
=> ~640-690 B/ns aggregate, above the ~430 B/ns DMA drain, so the DVE
does nothing but the 120 pixel copies.

Ramp: the fast-start slice is split in two overlapping parts -- S0AA
(j<12: pixels 0-7, only 1152 B/partition so it lands ~1.3us sooner and
the DVE starts that much earlier) then S0AB (j in [8,24): pixels 8-19).
Output pieces grow geometrically (1,2,3,4,6,9,12,3 pixels) so the SDMA
engines never idle after the input loads drain; the full tile-0 S
loads on the scalar HWDGE ring in two 2112B-descriptor halves so it
streams concurrently with the first output pieces.  Tiles 1-2 use host-compacted rows (R12) shifted
on-device, ALL eight byte shifts per tile as u8 copies on the
otherwise-idle Activation engine (its float path is exact for u8 only;
u32/u16 ACTIVATE copies corrupt data).  All output descriptors are
>= 1600 B, >= 3200 B after the first piece.

Known environment hazard (measured, not kernel-specific): under
cross-core HBM/fabric contention one SDMA engine of a core can drop to
~21 B/ns with ~12us periodicity, gating that core by +10-14us.  A solo
core never shows it; the staged baseline suffers it equally.
"""
import numpy as np

K = 8
MAX_SR = 3
B, H, W = 2, 192, 320
TP = MAX_SR + K // 2          # 7 pad per side
PW = W + 2 * TP               # 334
NCORES = 8
ROWS_PER_CORE = (B * H) // NCORES   # 48
WSEG = 40
NWSEG = W // WSEG             # 8
NSEG = ROWS_PER_CORE * NWSEG  # 384
NTILE = NSEG // 128           # 3

# sr=2 geometry
CV = 5
OSEG = WSEG * CV * CV * K * K   # 64000 output bytes per segment
PIXB = CV * CV * K * K          # 1600 output bytes per pixel
PIXW = PIXB // 4                # 400 u32 per pixel
DW = CV * K * K // 4            # 80 u32 per (pixel, dy)

NV = 12                       # source rows per segment (CV-1+K)
NU = 8                        # byte shifts u = phi+dx
NJ = 44                       # shifted sub-row bytes
SEGB = NV * NU * NJ           # 4224 S bytes per segment
AA_NJ = 12                    # fast-start A: j<12 (pixels 0-7)
AA_B = NV * NU * AA_NJ        # 1152
AB_NJ = 16                    # fast-start B: j in [8,24) (pixels 8-19)
AB_B = NV * NU * AB_NJ        # 1536
RJ = 56                       # compact row bytes (covers u+j <= 50)
RB = NV * RJ                  # 672 compact bytes per segment

# output piece sizes (pixels) per tile: geometric ramp for tile 0,
# full 20-px w-chunks for tiles 1-2.
PIECES_T0 = (1, 1, 2, 3, 4, 6, 9, 11, 3)
PIECES_T12 = (20, 20)

_PROG_CACHE = {}


def _make_host_arrays(x, sr):
    """x: (B,1,H,W) f32 -> per-core dict of host-prepped u8 arrays."""
    off = MAX_SR - sr
    P = np.pad(x[:, 0], ((0, 0), (TP, TP), (TP, TP))).astype(np.uint8)
    cores = []
    st = np.lib.stride_tricks.as_strided
    for c in range(NCORES):
        b = (c * ROWS_PER_CORE) // H
        h0 = (c * ROWS_PER_CORE) % H
        flat = np.ascontiguousarray(P[b]).reshape(-1)
        base = (h0 + off) * PW + off
        # S: tile-0 segments fully shifted: (r, s, v, u, j)
        s = st(flat[base:], shape=(16, NWSEG, NV, NU, NJ),
               strides=(PW, WSEG, PW, 1, 1))
        s = np.ascontiguousarray(s).reshape(128, SEGB)
        # S0a split: slice A (j<12, pixels 0-7) lands first so the DVE
        # starts ~1us earlier; slice B (j in [8,24), pixels 8-19)
        # follows.  They overlap by 4 bytes/row so no pixel's u32 reads
        # straddle the boundary.
        s0a_a = st(flat[base:], shape=(16, NWSEG, NV, NU, AA_NJ),
                   strides=(PW, WSEG, PW, 1, 1))
        s0a_a = np.ascontiguousarray(s0a_a).reshape(128, AA_B)
        s0a_b = st(flat[base + 8:], shape=(16, NWSEG, NV, NU, AB_NJ),
                   strides=(PW, WSEG, PW, 1, 1))
        s0a_b = np.ascontiguousarray(s0a_b).reshape(128, AB_B)
        # R12: compact un-shifted rows for tiles 1,2: (t, r, s, v, j)
        r12 = st(flat[base + 16 * PW:], shape=(2, 16, NWSEG, NV, RJ),
                 strides=(16 * PW, PW, WSEG, PW, 1))
        r12 = np.ascontiguousarray(r12.transpose(1, 2, 0, 3, 4)) \
            .reshape(128, 2 * RB)
        cores.append({"s0a_a": s0a_a, "s0a_b": s0a_b, "s": s, "r12": r12})
    return cores


def _build_program(sr):
    import concourse.bass as bass
    import concourse.bacc as bacc
    import concourse.mybir as mybir
    from concourse import tile

    u8 = mybir.dt.uint8
    u16 = mybir.dt.uint16
    u32 = mybir.dt.uint32
    nc = bacc.Bacc("TRN2", debug=False)
    s0aa_in = nc.declare_dram_parameter("s0a_a", [128, AA_B], u8,
                                        isOutput=False)
    s0ab_in = nc.declare_dram_parameter("s0a_b", [128, AB_B], u8,
                                        isOutput=False)
    s_in = nc.declare_dram_parameter("s", [128, SEGB], u8, isOutput=False)
    r12_in = nc.declare_dram_parameter("r12", [128, 2 * RB], u8,
                                       isOutput=False)
    out = nc.declare_dram_parameter("out", [NSEG * OSEG], u8, isOutput=True)

    with tile.TileContext(nc) as tc:
        with tc.tile_pool(name="spool", bufs=1) as sp, \
             tc.tile_pool(name="tpool", bufs=1) as tp:
            S0AA = sp.tile([128, AA_B], u8)
            S0AB = sp.tile([128, AB_B], u8)
            S0 = sp.tile([128, SEGB], u8)
            S1 = sp.tile([128, SEGB], u8)
            S2 = sp.tile([128, SEGB], u8)
            R12 = sp.tile([128, 2 * RB], u8)

            # input DMAs: fast-start slice + compact rows on the SP ring;
            # tile-0 full S on the scalar HWDGE ring (2112B descriptors)
            # so it streams while the first output pieces drain.
            nc.sync.dma_start(S0AA[:, :], s0aa_in[:, :])
            nc.sync.dma_start(S0AB[:, :], s0ab_in[:, :])
            nc.sync.dma_start(R12[:, :], r12_in[:, :])
            half = SEGB // 2
            nc.scalar.dma_start(S0[:, 0:half], s_in[:, 0:half])
            nc.scalar.dma_start(S0[:, half:SEGB], s_in[:, half:SEGB])

            S_tiles = (S0, S1, S2)

            def build_s_odd(t):
                """Odd byte shifts R12 -> S[t] on the Activation engine."""
                s8 = S_tiles[t][:]
                r8 = R12[:]
                for u in (1, 3, 5, 7):
                    src = bass.AP(r8.tensor, (t - 1) * RB + u,
                                  [[2 * RB, 128], [RJ, NV], [1, NJ]])
                    dst = bass.AP(s8.tensor, u * NJ,
                                  [[SEGB, 128], [NU * NJ, NV], [1, NJ]])
                    nc.scalar.copy(dst, src)

            def build_s_even(t):
                """Even byte shifts R12 -> S[t], also on the Activation
                engine as u8 copies (the ACT float path is exact for u8
                only -- u32/u16 ACTIVATE copies corrupt data), keeping
                the DVE dedicated to output expansion."""
                s8 = S_tiles[t][:]
                r8 = R12[:]
                for u in (0, 2, 4, 6):
                    src = bass.AP(r8.tensor, (t - 1) * RB + u,
                                  [[2 * RB, 128], [RJ, NV], [1, NJ]])
                    dst = bass.AP(s8.tensor, u * NJ,
                                  [[SEGB, 128], [NU * NJ, NV], [1, NJ]])
                    nc.scalar.copy(dst, src)

            def copy_px(T, slot, t, px):
                """One dy-fused copy: all (dy,ky,dx,kx) of pixel px of
                tile t into T at pixel-slot `slot` (400 u32/partition)."""
                a, phi = px // 4, px % 4
                if t == 0 and px < 8:
                    stile, su, pp, a0 = S0AA, AA_NJ // 4, AA_B // 4, 0
                elif t == 0 and px < 20:
                    stile, su, pp, a0 = S0AB, AB_NJ // 4, AB_B // 4, 2
                else:
                    stile, su, pp, a0 = S_tiles[t], NJ // 4, SEGB // 4, 0
                sv = NU * su
                s32 = stile[:].bitcast(u32)
                t32 = T[:].bitcast(u32)
                src = bass.AP(s32.tensor, phi * su + (a - a0),
                              [[pp, 128], [sv, CV], [sv, K], [su, CV],
                               [1, 2]])
                dst = bass.AP(t32.tensor, slot * PIXW,
                              [[20 * PIXW, 128], [DW, CV], [2, K],
                               [K * K // 4, CV], [1, 2]])
                nc.vector.tensor_copy(dst, src)

            # All shifts on ACT, as soon as R12 lands (after its two
            # s-half dma triggers).  Tile 1 completes first so the t1
            # expansion is never gated.
            build_s_odd(1)
            build_s_even(1)
            build_s_odd(2)
            build_s_even(2)

            def piece(t, p0, n):
                T = tp.tile([128, 20 * PIXB], u8, bufs=5, name="Tst")
                for i in range(n):
                    copy_px(T, i, t, p0 + i)
                nc.sync.dma_start(
                    bass.AP(out.ap().tensor, t * 128 * OSEG + p0 * PIXB,
                            [[OSEG, 128], [1, n * PIXB]]),
                    T[0:128, 0:n * PIXB])

            p0 = 0
            for n in PIECES_T0:
                piece(0, p0, n)
                p0 += n
            for t in (1, 2):
                p0 = 0
                for n in PIECES_T12:
                    piece(t, p0, n)
                    p0 += n
    nc.compile()
    return nc


def _numpy_fallback(x, sr):
    cv = 2 * sr + 1
    off = MAX_SR - sr
    P = np.pad(x[:, 0], ((0, 0), (TP, TP), (TP, TP))).astype(np.uint8)
    out = np.empty((B, H, W, cv * cv, K * K), np.uint8)
    for dy in range(cv):
        for dx in range(cv):
            for ky in range(K):
                for kx in range(K):
                    out[:, :, :, dy * cv + dx, ky * K + kx] = \
                        P[:, off + dy + ky:off + dy + ky + H,
                          off + dx + kx:off + dx + kx + W]
    return out


def kernel(inputs, search_range):
    from concourse.bass_utils import run_bass_kernel_spmd

    x = np.asarray(inputs, dtype=np.float32)
    sr = int(np.asarray(search_range))
    if sr != 2 or x.shape != (B, 1, H, W):
        return _numpy_fallback(x, sr)

    if sr not in _PROG_CACHE:
        _PROG_CACHE[sr] = _build_program(sr)
    nc = _PROG_CACHE[sr]

    host = _make_host_arrays(x, sr)
    res = run_bass_kernel_spmd(nc, host, list(range(NCORES)))
    outs = [np.asarray(res.results[c]["out"]) for c in range(NCORES)]
    return np.concatenate(outs).reshape(B, H, W, CV * CV, K * K)
